# revision 6
# baseline (speedup 1.0000x reference)
"""Bahdanau attention forward on 8 Trainium2 NeuronCores (data-parallel).

Per-core pipeline, batch rows on partitions, G=4 tiles (512 rows) per step:
  1. cast-DMA loads fp32->fp16 (value 2.5MB + query per group)
  2. h = v * q_bcast * W12rep  (two fp16 2x-mode DVE tensor_tensor ops;
     W12[w,e] = W1[e]*W2[w,e] host-folded, replicated fp16 constant)
  3. s = tanh(h) in place on ScalarE
  4. scores via custom DVE op ANT_MUL_SSCAN: a global inclusive cumsum of
     s*W3 along the stream (fp32 feedback register); per-w segment sums are
     recovered as differences of consecutive row-end elements
  5. softmax over W in fp32 (no max-subtraction needed: |scores| < ~50)
  6. context = sum_w a*v with the same cumsum op along the (e,w) stream,
     segment sums again via row-end differences (fp32 scan output)
  7. fp32 store
"""

import numpy as np

B, W, E = 65536, 20, 128
N_CORES = 8
B_CORE = B // N_CORES
G = 4

_CACHE = {}


def _register_scan_op():
    """Register a custom DVE op: global inclusive cumsum of in0*in1.

    The Scan keeps an fp32 feedback register and streams at 1 elem/cycle,
    fusing the elementwise multiply and the reduction into one DVE pass.
    It does NOT reset at subdim row boundaries (only PageIdx counters do),
    so callers recover per-row segment sums as differences of consecutive
    row-end elements of the fp32 cumsum output.
    """
    import re

    import concourse.dve_ops as dops
    from concourse import dve_spec as ds

    for o in dops.OPS:
        if o.name == "ANT_MUL_SSCAN":
            return o

    def _ref(in0, in1, c0, c1, c2):
        x = in0.astype(np.float32) * in1.astype(np.float32)
        return np.cumsum(x, axis=-1)

    spec = ds.Spec(
        body=ds.Scan(ds.AluOp.ADD, ds.Src0 * ds.Src1), reference=_ref
    )
    op = dops.DveOp("ANT_MUL_SSCAN", spec, subdim=True, uops_sha={})
    dops.OPS.append(op)
    dops._SUB_OPCODE_FOR_NAME[op.name] = dops._CUSTOM_DVE_ROW_BASE + len(dops.OPS) - 1
    for ver in ("v3", "v4"):
        try:
            op.compile(ver)
        except ValueError as e:
            m = re.search(r'"([0-9a-f]{16})"', str(e))
            if not m:
                raise
            op.uops_sha[ver] = m.group(1)
            op.compile(ver)
    return op


def _build(b_core: int, reps: int = 1):
    import sys

    if "/opt/trn_rl_repo" not in sys.path:
        sys.path.insert(0, "/opt/trn_rl_repo")
    import concourse.bacc as bacc
    import concourse.mybir as mybir
    import concourse.tile as tile

    f16 = mybir.dt.float16
    f32 = mybir.dt.float32
    WE = W * E
    n_tiles = b_core // 128
    n_groups = n_tiles // G
    assert b_core % (128 * G) == 0

    sscan = _register_scan_op()

    nc = bacc.Bacc(
        "TRN2",
        target_bir_lowering=False,
        debug=False,
        enable_asserts=False,
        num_devices=N_CORES,
    )

    value_d = nc.dram_tensor("value", [b_core, W, E], f32, kind="ExternalInput").ap()
    query_d = nc.dram_tensor("query", [b_core, E], f32, kind="ExternalInput").ap()
    w12_d = nc.dram_tensor("w12rep", [128, WE], f16, kind="ExternalInput").ap()
    w3_d = nc.dram_tensor("w3rep", [128, WE], f16, kind="ExternalInput").ap()
    eye_d = nc.dram_tensor("eye", [128, E], f16, kind="ExternalInput").ap()
    ctx_d = nc.dram_tensor("ctx", [b_core, E], f32, kind="ExternalOutput").ap()

    value_f = value_d.rearrange("b w e -> b (w e)")

    mult = mybir.AluOpType.mult
    add = mybir.AluOpType.add
    AXX = mybir.AxisListType.X
    Tanh = mybir.ActivationFunctionType.Tanh
    Exp = mybir.ActivationFunctionType.Exp
    Copy = mybir.ActivationFunctionType.Copy

    with tile.TileContext(nc) as tc:
        with (
            tc.tile_pool(name="consts", bufs=1) as cpool,
            tc.tile_pool(name="vbuf", bufs=2) as vpool,
            tc.tile_pool(name="hbuf", bufs=2) as hpool,
            tc.tile_pool(name="qbuf", bufs=2) as qpool,
            tc.tile_pool(name="small", bufs=2) as spool,
            tc.tile_pool(name="diag", bufs=2) as dpool,
            tc.tile_pool(name="ctxbuf", bufs=2) as opool,
            tc.tile_pool(name="psum", bufs=2, space="PSUM") as ppool,
        ):
            w12 = cpool.tile([128, WE], f16, tag="w12")
            nc.sync.dma_start(w12[:], w12_d)
            w3 = cpool.tile([128, WE], f16, tag="w3")
            nc.sync.dma_start(w3[:], w3_d)
            eye = cpool.tile([128, E], f16, tag="eye")
            nc.sync.dma_start(eye[:], eye_d)

            w12b = w12[:].unsqueeze(1).broadcast_to([128, G, WE])
            w3b = w3[:].unsqueeze(1).broadcast_to([128, G, WE])
            eyeb = eye[:].unsqueeze(1).broadcast_to([128, W, E])

            for gi in range(n_groups * reps):
                gi = gi % n_groups
                rows = slice(gi * G * 128, (gi + 1) * G * 128)

                VG = vpool.tile([128, G, WE], f16)
                nc.gpsimd.dma_start(
                    VG[:], value_f[rows, :].rearrange("(g p) c -> p g c", p=128)
                )
                V4 = VG[:].rearrange("p g (w e) -> p g w e", w=W)

                QG = qpool.tile([128, G, E], f16)
                nc.gpsimd.dma_start(
                    QG[:], query_d[rows, :].rearrange("(g p) e -> p g e", p=128)
                )

                HG = hpool.tile([128, G, WE], f16)
                H4 = HG[:].rearrange("p g (w e) -> p g w e", w=W)
                qb = QG[:].unsqueeze(2).broadcast_to([128, G, W, E])
                nc.vector.tensor_tensor(H4, V4, qb, mult)
                nc.vector.tensor_tensor(HG[:], HG[:], w12b, mult)
                nc.scalar.activation(HG[:], HG[:], Tanh)

                w3_3 = w3[:].rearrange("p (w e) -> p w e", w=W)
                scores = spool.tile([128, G, W], f32, tag="scores")
                for t in range(G):
                    ss = dpool.tile([128, W, E], f32, tag="sscan")
                    nc.vector._custom_dve(
                        sscan, out=ss[:], in0=H4[:, t], in1=w3_3
                    )
                    # global cumsum: segment sums = diffs at row-end positions
                    cend = ss[:][:, :, E - 1]  # [128, W]
                    nc.scalar.copy(scores[:, t, 0:1], cend[:, 0:1])
                    nc.vector.tensor_tensor(
                        scores[:, t, 1:W], cend[:, 1:W], cend[:, 0 : W - 1],
                        mybir.AluOpType.subtract,
                    )
                e32 = spool.tile([128, G, W], f32, tag="e32")
                nc.scalar.activation(e32[:], scores[:], Exp)
                denom = spool.tile([128, G], f32, tag="denom")
                nc.vector.tensor_reduce(denom[:], e32[:], AXX, add)
                rec = spool.tile([128, G], f32, tag="rec")
                nc.vector.reciprocal(rec[:], denom[:])
                a32 = spool.tile([128, G, W], f32, tag="a32")
                recb = rec[:].unsqueeze(2).broadcast_to([128, G, W])
                nc.vector.tensor_tensor(a32[:], e32[:], recb, mult)

                # ctx via global cumsum of a*v along the (e,w) stream:
                # ctx[:,e] = csum[e*W+W-1] - csum[(e-1)*W+W-1]
                cout = opool.tile([128, G, E], f32)
                for t in range(G):
                    vt = V4[:, t].transpose([0, 2, 1])  # [128, E, W], w strided
                    ab = a32[:, t, :].unsqueeze(1).broadcast_to([128, E, W])
                    cs = dpool.tile([128, E, W], f32, tag="cscan")
                    nc.vector._custom_dve(sscan, out=cs[:], in0=vt, in1=ab)
                    cend = cs[:][:, :, W - 1]  # [128, E]
                    nc.scalar.copy(cout[:, t, 0:1], cend[:, 0:1])
                    nc.vector.tensor_tensor(
                        cout[:, t, 1:E], cend[:, 1:E], cend[:, 0 : E - 1],
                        mybir.AluOpType.subtract,
                    )

                nc.sync.dma_start(
                    ctx_d[rows, :].rearrange("(g p) e -> p g e", p=128), cout[:]
                )

    nc.compile()
    return nc


def _get_nc(b_core: int):
    if b_core not in _CACHE:
        _CACHE[b_core] = _build(b_core)
    return _CACHE[b_core]


def _host_weights(W1, W2, W3):
    w12 = (W1.astype(np.float32)[0][None, :] * W2.astype(np.float32)).reshape(-1)
    w12rep = np.broadcast_to(w12, (128, W * E)).astype(np.float16)
    w3rep = np.broadcast_to(
        W3.astype(np.float32).reshape(-1), (128, W * E)
    ).astype(np.float16)
    eye = np.eye(128, dtype=np.float16)
    return (
        np.ascontiguousarray(w12rep),
        np.ascontiguousarray(w3rep),
        np.ascontiguousarray(eye),
    )


def kernel(query, value, W1, W2, W3):
    import sys

    if "/opt/trn_rl_repo" not in sys.path:
        sys.path.insert(0, "/opt/trn_rl_repo")
    from concourse.bass_utils import run_bass_kernel_spmd

    query = np.asarray(query, dtype=np.float32)
    value = np.asarray(value, dtype=np.float32)
    w12rep, w3rep, eye = _host_weights(
        np.asarray(W1), np.asarray(W2), np.asarray(W3)
    )

    nc = _get_nc(B_CORE)
    in_maps = []
    for c in range(N_CORES):
        rows = slice(c * B_CORE, (c + 1) * B_CORE)
        in_maps.append(
            {
                "value": np.ascontiguousarray(value[rows]),
                "query": np.ascontiguousarray(query[rows]),
                "w12rep": w12rep,
                "w3rep": w3rep,
                "eye": eye,
            }
        )

    res = run_bass_kernel_spmd(nc, in_maps, list(range(N_CORES)))
    out = np.concatenate([res.results[c]["ctx"] for c in range(N_CORES)], axis=0)
    return out.astype(np.float32)
